# revision 1
# baseline (speedup 1.0000x reference)
"""Trainium2 Bass kernel for BinaryReflectanceGate (gnn_message_passing).

Math (reference):
    feat = [pos, refl]                    # [N,4]
    h1 = relu(feat @ W1 + b1)             # [N,16]
    h2 = relu(h1 @ W2 + b2)               # [N,16]
    smax = segment_max(h2, batch, B=64)   # [64,16]
    logits = smax @ Wg + bg               # [64,2]
    gate = softmax((logits + gumbels)/tau)[:, 1]
    out = gate[batch] * refl              # [N]

Kernel strategy (8 cores, data parallel over whole samples):
  - batch is sorted; core k owns segments [8k, 8k+8).  Each segment is
    padded to a uniform capacity S_cap (multiple of 4096 -- tile-aligned,
    so segment boundaries never split a tile across partitions); pad
    points replicate the segment's first point so the max is unchanged.
  - 8-group interleaved layout: 8 groups of 512 points share the 128
    partitions (partition = 4*g+f for layer-1 rhs, 16*g+ch for hidden),
    with block-diagonal weights -> full 128-wide contraction per matmul.
  - fp32r matmuls: 1 cycle/row on the PE at ~2e-4 relative error.
  - softmax over 2 classes == sigmoid of the logit difference:
        gate = sigmoid(smax @ (Wg[:,1]-Wg[:,0]) + (bg[1]-bg[0]) + gdel)
    where gdel = gumbels[:,1]-gumbels[:,0]  (tau = 1).
  - relu/b2 are deferred past the segment max (both monotone):
        relu(max(z2) + b2) == max(relu(z2 + b2)).
  - per-segment finishers are emitted right after the segment's last
    z2 chunk, so the gate computation and the reflectance scaling of
    earlier segments hide under the main loop of later segments.
"""
import sys
sys.path.insert(0, "/opt/trn_rl_repo")

import numpy as np
import concourse.bass as bass
import concourse.bacc as bacc
from concourse import mybir
from concourse.tile import TileContext
from concourse.bass_utils import run_bass_kernel_spmd

N = 4_194_304
B = 64
H = 16
NCORES = 8
SEGS_PER_CORE = B // NCORES  # 8
TILE_PTS = 4096              # points per [*,512] matmul tile (8 groups x 512)
CHUNK_PTS = 8192             # points per z-chunk ([128,1024] psum)

F32 = mybir.dt.float32
F32R = mybir.dt.float32r


def _regions_for_chunk(k, S_cap):
    """Reduce regions for chunk k (points [k*8192, (k+1)*8192)).

    Returns list of (seg_local, p_lo, p_hi, c_lo, c_hi) ordered by segment.
    Column c of the chunk maps to tile t = 2k + (c >= 512), group g = row//16,
    point t*4096 + g*512 + (c % 512).
    """
    p0 = k * CHUNK_PTS
    s0 = p0 // S_cap
    s1 = (p0 + CHUNK_PTS - 1) // S_cap
    if s0 == s1:
        return [(s0, 0, 128, 0, 1024)]
    b = s1 * S_cap
    out = []
    for half, t in enumerate((2 * k, 2 * k + 1)):
        t0 = t * TILE_PTS
        c_lo, c_hi = 512 * half, 512 * (half + 1)
        if b <= t0:
            out.append((s1, 0, 128, c_lo, c_hi))
        elif b >= t0 + TILE_PTS:
            out.append((s0, 0, 128, c_lo, c_hi))
        else:
            m = (b - t0) // 512
            out.append((s0, 0, 16 * m, c_lo, c_hi))
            out.append((s1, 16 * m, 128, c_lo, c_hi))
    out.sort(key=lambda r: r[0])
    return out


def _build_program(S_cap):
    T = S_cap // 512          # [*,512] tiles per core
    T4 = T // 4               # [128,512] feature blocks per core
    NCHUNK = T // 2
    C = S_cap // 16           # columns of refl/out [128, C]
    W = S_cap // 128          # columns per segment in refl/out
    # persistent refl/out SBUF buffers only when they fit comfortably;
    # otherwise stream the reflectance scaling in pieces (extreme skew)
    big_sbuf_ok = C <= 8192

    # slot assignment for minis + last chunk of each segment
    chunk_regions = []        # per chunk: list of (seg, p_lo, p_hi, c_lo, c_hi, slot)
    seg_slots = [[] for _ in range(SEGS_PER_CORE)]
    seg_last_chunk = [0] * SEGS_PER_CORE
    nslot = 0
    for k in range(NCHUNK):
        regs = []
        for (s, p_lo, p_hi, c_lo, c_hi) in _regions_for_chunk(k, S_cap):
            regs.append((s, p_lo, p_hi, c_lo, c_hi, nslot))
            seg_slots[s].append(nslot)
            seg_last_chunk[s] = k
            nslot += 1
        chunk_regions.append(regs)
    for s in range(SEGS_PER_CORE):
        sl = seg_slots[s]
        assert sl == list(range(sl[0], sl[-1] + 1)), "slots not contiguous"

    nc = bacc.Bacc()

    feat_d = nc.declare_dram_parameter("feat", [T4, 128, 512], F32R, isOutput=False)
    refl_d = nc.declare_dram_parameter("refl", [128, C], F32, isOutput=False)
    gdb_d = nc.declare_dram_parameter("gdb", [1, 8], F32, isOutput=False)
    w1_d = nc.declare_dram_parameter("w1q", [128, 128], F32R, isOutput=False)
    w1x_d = nc.declare_dram_parameter("w1x", [128, 128], F32R, isOutput=False)
    w2_d = nc.declare_dram_parameter("w2b", [128, 128], F32R, isOutput=False)
    b1_d = nc.declare_dram_parameter("b1r", [128, 1], F32, isOutput=False)
    b2_d = nc.declare_dram_parameter("b2r", [1, 16], F32, isOutput=False)
    wd_d = nc.declare_dram_parameter("wdr", [1, 16], F32, isOutput=False)
    id_d = nc.declare_dram_parameter("ident", [128, 128], F32, isOutput=False)
    out_d = nc.declare_dram_parameter("out", [128, C], F32, isOutput=True)

    with TileContext(nc) as tc:
        with tc.tile_pool(name="consts", bufs=1) as consts, \
             tc.tile_pool(name="big", bufs=1) as big, \
             tc.tile_pool(name="feat", bufs=4) as featp, \
             tc.tile_pool(name="h1", bufs=4) as h1p, \
             tc.tile_pool(name="fin", bufs=1) as fin, \
             tc.tile_pool(name="mul", bufs=4) as mulp, \
             tc.tile_pool(name="z1", bufs=2, space="PSUM") as z1p, \
             tc.tile_pool(name="z2", bufs=2, space="PSUM") as z2p:

            w1t = consts.tile([128, 128], F32R)
            w1xt = consts.tile([128, 128], F32R)
            w2t = consts.tile([128, 128], F32R)
            b1t = consts.tile([128, 1], F32)
            b2r = consts.tile([1, 16], F32)
            wdr = consts.tile([1, 16], F32)
            gdbt = consts.tile([1, 8], F32)
            ident = consts.tile([128, 128], F32)
            nc.sync.dma_start(out=w1t, in_=w1_d[:])

            if big_sbuf_ok:
                reflt = big.tile([128, C], F32)
                outt = big.tile([128, C], F32)

            minis = fin.tile([128, nslot], F32)
            nc.gpsimd.memset(minis, -1e30)
            # preload the ACT table set (sigmoid set also contains relu)
            # before the first real activation, hiding the ~2.7us load
            # under the initial DMAs
            preact = fin.tile([1, 1], F32)
            nc.vector.memset(preact, 0.0)
            nc.scalar.activation(out=preact, in_=preact[:],
                                 func=mybir.ActivationFunctionType.Sigmoid,
                                 bias=0.0, scale=1.0)
            nc.scalar.activation(out=preact, in_=preact[:],
                                 func=mybir.ActivationFunctionType.Relu,
                                 bias=0.0, scale=1.0)

            seg_rows = {}

            def finish_segment_stage1(s):
                # group-combine prep, emitted right after the segment's
                # last z2 chunk: fold the mini slots and transpose to a row
                lo, hi = seg_slots[s][0], seg_slots[s][-1] + 1
                red = fin.tile([128, 1], F32, tag=f"red{s}")
                nc.vector.reduce_max(red, minis[:, lo:hi],
                                     axis=mybir.AxisListType.X)
                # transpose to a row: tp[0, 16g+ch] = red[16g+ch]
                tp = z2p.tile([1, 128], F32, tag="z2c")
                nc.tensor.transpose(tp, red[:], ident[:])
                row16 = fin.tile([1, 16], F32, tag=f"row{s}")
                nc.vector.reduce_max(
                    row16, tp.rearrange("one (g ch) -> one ch g", g=8),
                    axis=mybir.AxisListType.X)
                seg_rows[s] = row16

            def finish_segment_stage2(s):
                # gate + reflectance scaling; emitted a couple of chunks
                # later so the gate chain never stalls the DVE stream
                row16 = seg_rows[s]
                # relu(smax + b2), dot with wd; the last segment's chain is
                # tail-exposed, so keep it on DVE (fewer cross-engine hops)
                eng = nc.vector if s == SEGS_PER_CORE - 1 else nc.gpsimd
                srel = fin.tile([1, 16], F32, tag=f"srel{s}")
                eng.tensor_add(srel, row16, b2r[:])
                eng.tensor_scalar_max(srel, srel, 0.0)
                eng.tensor_mul(srel, srel, wdr[:])
                logit = fin.tile([1, 1], F32, tag=f"lg{s}")
                nc.vector.reduce_sum(logit, srel, axis=mybir.AxisListType.X)
                gate1 = fin.tile([1, 1], F32, tag=f"g{s}")
                nc.scalar.activation(out=gate1, in_=logit[:],
                                     func=mybir.ActivationFunctionType.Sigmoid,
                                     bias=gdbt[0:1, s:s + 1], scale=1.0)
                gbc = fin.tile([128, 1], F32, tag=f"gb{s}")
                nc.gpsimd.partition_broadcast(gbc, gate1[:])
                if big_sbuf_ok:
                    nc.gpsimd.tensor_scalar_mul(
                        outt[:, W * s:W * (s + 1)],
                        reflt[:, W * s:W * (s + 1)],
                        gbc[:, 0:1])
                    nc.sync.dma_start(out=out_d[:, W * s:W * (s + 1)],
                                      in_=outt[:, W * s:W * (s + 1)])
                else:
                    PIECE = 2048
                    for c0 in range(0, W, PIECE):
                        cw = min(PIECE, W - c0)
                        lo_c = W * s + c0
                        rt = mulp.tile([128, PIECE], F32, tag="rt")
                        nc.sync.dma_start(out=rt[:, :cw],
                                          in_=refl_d[:, lo_c:lo_c + cw])
                        ot = mulp.tile([128, PIECE], F32, tag="ot")
                        nc.gpsimd.tensor_scalar_mul(ot[:, :cw], rt[:, :cw],
                                                    gbc[:, 0:1])
                        nc.sync.dma_start(out=out_d[:, lo_c:lo_c + cw],
                                          in_=ot[:, :cw])

            refl_dma_done = set()
            fq2 = None
            for q in range(T4):
                # q==0 loads a single block so the first matmul starts as
                # early as possible; later blocks load in 512KB pairs
                if q == 0:
                    fq2 = featp.tile([128, 1024], F32R, tag="fq")
                    nc.sync.dma_start(out=fq2[:, 0:512], in_=feat_d[0])
                    fq = fq2[:, 0:512]
                elif (q - 1) % 2 == 0:
                    fq2 = featp.tile([128, 1024], F32R, tag="fq")
                    if q + 1 < T4:
                        nc.sync.dma_start(
                            out=fq2,
                            in_=bass.AP(tensor=feat_d.ap().tensor, offset=q * 65536,
                                        ap=[[512, 128], [65536, 2], [1, 512]]))
                    else:
                        nc.sync.dma_start(out=fq2[:, 0:512], in_=feat_d[q])
                    fq = fq2[:, 0:512]
                else:
                    fq = fq2[:, 512:1024]
                if q == 0:
                    # critical consts, interleaved behind the first
                    # feature tile so the first matmul starts early
                    nc.sync.dma_start(out=b1t, in_=b1_d[:])
                    nc.sync.dma_start(out=w2t, in_=w2_d[:])
                    nc.sync.dma_start(out=w1xt, in_=w1x_d[:])
                if q == 1:
                    # non-critical consts, needed first by segment 0's
                    # finisher -- keep them off the critical DMA path
                    nc.sync.dma_start(out=ident, in_=id_d[:])
                    nc.sync.dma_start(out=b2r, in_=b2_d[:])
                    nc.sync.dma_start(out=wdr, in_=wd_d[:])
                    nc.sync.dma_start(out=gdbt, in_=gdb_d[:])
                # spread the per-segment reflectance loads across the loop
                s_here = (q * TILE_PTS * 4) // S_cap
                if big_sbuf_ok and s_here not in refl_dma_done:
                    refl_dma_done.add(s_here)
                    nc.sync.dma_start(
                        out=reflt[:, W * s_here:W * (s_here + 1)],
                        in_=refl_d[:, W * s_here:W * (s_here + 1)])
                for half in range(2):
                    z1c = z1p.tile([128, 1024], F32, tag="z1c")
                    for j in range(2):
                        a = 2 * half + j
                        if a < 3:
                            nc.tensor.matmul(
                                z1c[:, 512 * j:512 * (j + 1)],
                                lhsT=w1t[32 * a:32 * (a + 1), :],
                                rhs=fq[32 * a:32 * (a + 1), :],
                                start=True, stop=True)
                        else:
                            # base partition 96 is illegal; contract K=64
                            # from base 64 with zero top half in the weights
                            nc.tensor.matmul(
                                z1c[:, 512 * j:512 * (j + 1)],
                                lhsT=w1xt[64:128, :],
                                rhs=fq[64:128, :],
                                start=True, stop=True)
                    h1c = h1p.tile([128, 1024], F32R, tag="h1c")
                    nc.scalar.activation(
                        out=h1c, in_=z1c[:],
                        func=mybir.ActivationFunctionType.Relu,
                        bias=b1t[:, 0:1], scale=1.0)
                    z2c = z2p.tile([128, 1024], F32, tag="z2c")
                    for j in range(2):
                        nc.tensor.matmul(
                            z2c[:, 512 * j:512 * (j + 1)],
                            lhsT=w2t[:],
                            rhs=h1c[:, 512 * j:512 * (j + 1)],
                            start=True, stop=True)
                    k = 2 * q + half
                    for (s, p_lo, p_hi, c_lo, c_hi, slot) in chunk_regions[k]:
                        nc.vector.reduce_max(
                            minis[p_lo:p_hi, slot:slot + 1],
                            z2c[p_lo:p_hi, c_lo:c_hi],
                            axis=mybir.AxisListType.X)
                    for s in range(SEGS_PER_CORE):
                        if seg_last_chunk[s] == k:
                            finish_segment_stage1(s)
                        if seg_last_chunk[s] == k - 2:
                            finish_segment_stage2(s)
            for s in range(SEGS_PER_CORE):
                if seg_last_chunk[s] >= NCHUNK - 3:
                    finish_segment_stage2(s)

    nc.compile()
    return nc


_CACHE = {}


def _program(S_cap):
    if S_cap not in _CACHE:
        _CACHE[S_cap] = _build_program(S_cap)
    return _CACHE[S_cap]


def _prep_inputs(pos, reflectance, batch, gumbels, W1, b1, W2, b2, Wg, bg):
    pos = np.asarray(pos, np.float32)
    reflectance = np.asarray(reflectance, np.float32)
    batch = np.asarray(batch, np.int32)
    gumbels = np.asarray(gumbels, np.float32)
    W1, b1 = np.asarray(W1, np.float32), np.asarray(b1, np.float32)
    W2, b2 = np.asarray(W2, np.float32), np.asarray(b2, np.float32)
    Wg, bg = np.asarray(Wg, np.float32), np.asarray(bg, np.float32)

    bounds = np.searchsorted(batch, np.arange(B + 1), side="left")
    seg_len = np.diff(bounds)
    S_cap = int(-(-max(1, seg_len.max()) // 4096) * 4096)
    T = S_cap // 512
    T4 = T // 4
    C = S_cap // 16

    feat = np.concatenate([pos, reflectance[:, None]], axis=1)  # [N,4]

    # constants (shared across cores)
    w1q = np.zeros((128, 128), np.float32)
    w2b = np.zeros((128, 128), np.float32)
    for g in range(8):
        w2b[16 * g:16 * (g + 1), 16 * g:16 * (g + 1)] = W2
    for a in range(4):
        for g in range(8):
            w1q[32 * a + 4 * g:32 * a + 4 * (g + 1), 16 * g:16 * (g + 1)] = W1
    w1x = np.zeros((128, 128), np.float32)
    w1x[96:128] = w1q[96:128]
    b1r = np.tile(b1, 8)[:, None].astype(np.float32)
    b2r = b2[None, :].astype(np.float32)
    wdr = (Wg[:, 1] - Wg[:, 0])[None, :].astype(np.float32)
    ident = np.eye(128, dtype=np.float32)
    gdel = (bg[1] - bg[0]) + gumbels[:, 1] - gumbels[:, 0]  # [B]

    in_maps = []
    for core in range(NCORES):
        fpad = np.zeros((SEGS_PER_CORE, S_cap, 4), np.float32)
        rpad = np.zeros((SEGS_PER_CORE, S_cap), np.float32)
        for s in range(SEGS_PER_CORE):
            seg = SEGS_PER_CORE * core + s
            lo, hi = bounds[seg], bounds[seg + 1]
            n = hi - lo
            if n > 0:
                fpad[s, :n] = feat[lo:hi]
                fpad[s, n:] = feat[lo]        # replicate first point
                rpad[s, :n] = reflectance[lo:hi]
        # feat_host[q, 32a + 4g+f, c] = fpad[point (4q+a)*4096 + g*512 + c, f]
        fh = (fpad.reshape(S_cap * SEGS_PER_CORE, 4)
                  .reshape(T4, 4, 8, 512, 4)      # q, a, g, c, f
                  .transpose(0, 1, 2, 4, 3)       # q, a, g, f, c
                  .reshape(T4, 128, 512))
        rh = rpad.reshape(C, 128).T               # [128, C]
        gdb = gdel[SEGS_PER_CORE * core:SEGS_PER_CORE * (core + 1)][None, :]
        in_maps.append({
            "feat": np.ascontiguousarray(fh),
            "refl": np.ascontiguousarray(rh),
            "gdb": np.ascontiguousarray(gdb.astype(np.float32)),
            "w1q": w1q, "w1x": w1x, "w2b": w2b, "b1r": b1r,
            "b2r": b2r, "wdr": wdr, "ident": ident,
        })
    return in_maps, bounds, S_cap


_LAST_S_CAP = None


def _run(trace=False, **inputs):
    global _LAST_S_CAP
    in_maps, bounds, S_cap = _prep_inputs(**inputs)
    _LAST_S_CAP = S_cap
    nc = _program(S_cap)
    res = run_bass_kernel_spmd(nc, in_maps, list(range(NCORES)), trace=trace)
    out = np.empty(N, np.float32)
    for core in range(NCORES):
        o = res.results[core]["out"]              # [128, C]
        flat = o.T.reshape(SEGS_PER_CORE, S_cap)  # [s, S_cap]
        for s in range(SEGS_PER_CORE):
            seg = SEGS_PER_CORE * core + s
            lo, hi = bounds[seg], bounds[seg + 1]
            if hi > lo:
                out[lo:hi] = flat[s, :hi - lo]
    return out, res


def kernel(**inputs) -> np.ndarray:
    out, _ = _run(trace=False, **inputs)
    return out



# revision 4
# speedup vs baseline: 1.0326x; 1.0326x over previous
"""Trainium2 Bass kernel for BinaryReflectanceGate (gnn_message_passing).

Math (reference):
    feat = [pos, refl]                    # [N,4]
    h1 = relu(feat @ W1 + b1)             # [N,16]
    h2 = relu(h1 @ W2 + b2)               # [N,16]
    smax = segment_max(h2, batch, B=64)   # [64,16]
    logits = smax @ Wg + bg               # [64,2]
    gate = softmax((logits + gumbels)/tau)[:, 1]
    out = gate[batch] * refl              # [N]

Kernel v2 strategy (8 cores, data parallel over whole samples):
  - batch is sorted; core k owns segments [8k, 8k+8), each padded to a
    uniform S_cap (multiple of 1024; pad points replicate the segment's
    first point so the max is unchanged).
  - column-major group interleave: core-point p -> chunk k = p//8192,
    col c = (p%8192)//8, group g = p%8.  Chunks are [128, 1024] tiles
    (partition = 16g+ch) and segment boundaries fall on column
    boundaries inside a chunk, so boundary chunks split into column
    ranges (no double-cost partition splits).
  - bf16 matmuls (1 cycle/row): L1 via [32,512]-rhs block-diagonal
    weights (8 groups x 4 features), L2 via [128,128] block-diagonal.
  - relu/b1 fused in one ACT activation per chunk, bf16 h1 out.
  - relu/b2 deferred past the segment max (monotone).
  - the per-chunk segment-max runs on DVE (the only engine that can
    column-reduce PSUM).  ACT is cheaper per column than DVE, so a few
    chunks per core take a copy path instead: ACT copies z2 to SBUF,
    Pool (which cannot touch PSUM) max-folds the copy, and a tiny DVE
    reduce finishes the slot.  This shifts work from DVE (the
    bottleneck) onto ACT/Pool headroom.
  - softmax over 2 classes == sigmoid of the logit difference.
  - per-segment finishers: Pool folds minis slots pairwise, PE
    transposes, DVE does the 8-group row max + wd dot, ACT applies the
    sigmoid, Pool broadcasts the gate and scales the reflectance.
"""
import sys
sys.path.insert(0, "/opt/trn_rl_repo")

import numpy as np
import ml_dtypes
import concourse.bass as bass
import concourse.bacc as bacc
from concourse import mybir
from concourse.tile import TileContext
from concourse.bass_utils import run_bass_kernel_spmd

N = 4_194_304
B = 64
H = 16
NCORES = 8
SEGS_PER_CORE = B // NCORES    # 8
CHUNK_PTS = 8192               # points per [128,1024] z chunk
NPBF = ml_dtypes.bfloat16

F32 = mybir.dt.float32
BF16 = mybir.dt.bfloat16

# number of chunks routed ACT-copy + accum instead of DVE reduce
N_COPY = 0


def _chunk_regions(S_cap):
    """Per chunk: list of (seg, c_lo, c_hi).  Segment boundaries are at
    core-point s*S_cap; chunk k covers points [8192k, 8192(k+1)) with
    col c = (p - 8192k)//8."""
    nchunk = 8 * S_cap // CHUNK_PTS
    regions = []
    for k in range(nchunk):
        p0, p1 = k * CHUNK_PTS, (k + 1) * CHUNK_PTS
        s0, s1 = p0 // S_cap, (p1 - 1) // S_cap
        if s0 == s1:
            regions.append([(s0, 0, 1024)])
        else:
            m = (s1 * S_cap - p0) // 8
            regions.append([(s0, 0, m), (s1, m, 1024)])
    return regions


def _build_program(S_cap):
    assert S_cap % 1024 == 0
    NCHUNK = 8 * S_cap // CHUNK_PTS
    W = S_cap // 128            # refl/out columns per segment
    C = 8 * W                   # [128, C] refl/out

    regions = _chunk_regions(S_cap)
    # choose copy chunks: single-segment chunks away from segment ends
    seg_last_chunk = [0] * SEGS_PER_CORE
    for k, regs in enumerate(regions):
        for (s, _, _) in regs:
            seg_last_chunk[s] = k
    copy_ok = [k for k, regs in enumerate(regions)
               if len(regs) == 1 and k + 3 < seg_last_chunk[regs[0][0]] and k > 1]
    stride = max(1, len(copy_ok) // max(1, N_COPY))
    copy_chunks = set(copy_ok[stride // 2::stride][:N_COPY])

    # slot assignment (per region, sequential -> per-seg slots contiguous)
    chunk_slots = []            # per chunk: list of (seg, c_lo, c_hi, slot)
    seg_slots = [[] for _ in range(SEGS_PER_CORE)]
    nslot = 0
    for k, regs in enumerate(regions):
        out = []
        for (s, c_lo, c_hi) in regs:
            out.append((s, c_lo, c_hi, nslot))
            seg_slots[s].append(nslot)
            nslot += 1
        chunk_slots.append(out)
    for s in range(SEGS_PER_CORE):
        sl = seg_slots[s]
        assert sl == list(range(sl[0], sl[-1] + 1))

    nc = bacc.Bacc()

    feat_d = nc.declare_dram_parameter("feat", [NCHUNK * 32, 1024], BF16,
                                       isOutput=False)
    refl_d = nc.declare_dram_parameter("refl", [128, C], F32, isOutput=False)
    gdb_d = nc.declare_dram_parameter("gdb", [1, 8], F32, isOutput=False)
    w1_d = nc.declare_dram_parameter("w1b", [32, 128], BF16, isOutput=False)
    w2_d = nc.declare_dram_parameter("w2b", [128, 128], BF16, isOutput=False)
    b1_d = nc.declare_dram_parameter("b1r", [128, 1], F32, isOutput=False)
    b2_d = nc.declare_dram_parameter("b2r", [1, 16], F32, isOutput=False)
    wd_d = nc.declare_dram_parameter("wdr", [1, 16], F32, isOutput=False)
    id_d = nc.declare_dram_parameter("ident", [128, 128], F32, isOutput=False)
    out_d = nc.declare_dram_parameter("out", [128, C], F32, isOutput=True)

    with TileContext(nc) as tc:
        with tc.tile_pool(name="consts", bufs=1) as consts, \
             tc.tile_pool(name="big", bufs=1) as big, \
             tc.tile_pool(name="feat", bufs=3) as featp, \
             tc.tile_pool(name="h1", bufs=3) as h1p, \
             tc.tile_pool(name="zc", bufs=2) as zcp, \
             tc.tile_pool(name="fin", bufs=1) as fin, \
             tc.tile_pool(name="z1", bufs=2, space="PSUM") as z1p, \
             tc.tile_pool(name="z2", bufs=2, space="PSUM") as z2p:

            w1t = consts.tile([32, 128], BF16)
            w2t = consts.tile([128, 128], BF16)
            b1t = consts.tile([128, 1], F32)
            b2r = consts.tile([1, 16], F32)
            wdr = consts.tile([1, 16], F32)
            gdbt = consts.tile([1, 8], F32)
            ident = consts.tile([128, 128], F32)

            reflt = big.tile([128, C], F32)
            outt = big.tile([128, C], F32)

            minis = fin.tile([128, nslot], F32)
            nc.gpsimd.memset(minis, -1e30)
            # preload the ACT table set (sigmoid set also contains relu
            # and copy) before the first real activation
            preact = fin.tile([1, 1], F32)
            nc.vector.memset(preact, 0.0)
            nc.scalar.activation(out=preact, in_=preact[:],
                                 func=mybir.ActivationFunctionType.Sigmoid,
                                 bias=0.0, scale=1.0)
            nc.scalar.activation(out=preact, in_=preact[:],
                                 func=mybir.ActivationFunctionType.Relu,
                                 bias=0.0, scale=1.0)

            seg_red = {}

            def finish_segment_stage1(s):
                # fold this segment's minis slots into one column (DVE),
                # then transpose to a row (PE)
                lo, hi = seg_slots[s][0], seg_slots[s][-1] + 1
                red = fin.tile([128, 1], F32, tag=f"red{s}")
                nc.vector.reduce_max(red, minis[:, lo:hi],
                                     axis=mybir.AxisListType.X)
                tp = z2p.tile([1, 128], F32, tag="z2c")
                nc.tensor.transpose(tp, red[:], ident[:])
                seg_red[s] = tp

            def finish_segment_stage2(s):
                # 8-group row max + gate + reflectance scaling
                tp = seg_red[s]
                row16 = fin.tile([1, 16], F32, tag=f"row{s}")
                nc.vector.reduce_max(
                    row16, tp.rearrange("one (g ch) -> one ch g", g=8),
                    axis=mybir.AxisListType.X)
                srel = fin.tile([1, 16], F32, tag=f"srel{s}")
                nc.gpsimd.tensor_tensor(srel, row16, b2r[:],
                                        op=mybir.AluOpType.add)
                nc.gpsimd.tensor_scalar_max(srel, srel, 0.0)
                nc.gpsimd.tensor_mul(srel, srel, wdr[:])
                logit = fin.tile([1, 1], F32, tag=f"lg{s}")
                nc.vector.reduce_sum(logit, srel, axis=mybir.AxisListType.X)
                gate1 = fin.tile([1, 1], F32, tag=f"g{s}")
                nc.scalar.activation(out=gate1, in_=logit[:],
                                     func=mybir.ActivationFunctionType.Sigmoid,
                                     bias=gdbt[0:1, s:s + 1], scale=1.0)
                gbc = fin.tile([128, 1], F32, tag=f"gb{s}")
                nc.gpsimd.partition_broadcast(gbc, gate1[:])
                eng = nc.vector if s == SEGS_PER_CORE - 1 else nc.gpsimd
                eng.tensor_scalar_mul(outt[:, W * s:W * (s + 1)],
                                      reflt[:, W * s:W * (s + 1)],
                                      gbc[:, 0:1])
                nc.sync.dma_start(out=out_d[:, W * s:W * (s + 1)],
                                  in_=outt[:, W * s:W * (s + 1)])

            refl_dma_done = set()
            ft = None
            for k in range(NCHUNK):
                if k % 4 == 0:
                    nq = min(4, NCHUNK - k)
                    ft = featp.tile([32, 4096], BF16, tag="ft")
                    nc.sync.dma_start(
                        out=ft[:, 0:1024 * nq],
                        in_=bass.AP(tensor=feat_d.ap().tensor,
                                    offset=k * 32768,
                                    ap=[[1024, 32], [32768, nq], [1, 1024]]))
                if k == 0:
                    nc.sync.dma_start(out=w1t, in_=w1_d[:])
                    nc.sync.dma_start(out=b1t, in_=b1_d[:])
                    nc.sync.dma_start(out=w2t, in_=w2_d[:])
                if k == 1:
                    nc.sync.dma_start(out=ident, in_=id_d[:])
                    nc.sync.dma_start(out=b2r, in_=b2_d[:])
                    nc.sync.dma_start(out=wdr, in_=wd_d[:])
                    nc.sync.dma_start(out=gdbt, in_=gdb_d[:])
                # spread the reflectance loads across the loop (2 segs per DMA)
                s_here = (k * CHUNK_PTS) // (2 * S_cap)
                if s_here not in refl_dma_done:
                    refl_dma_done.add(s_here)
                    nc.sync.dma_start(
                        out=reflt[:, 2 * W * s_here:2 * W * (s_here + 1)],
                        in_=refl_d[:, 2 * W * s_here:2 * W * (s_here + 1)])

                fq = ft[:, 1024 * (k % 4):1024 * (k % 4 + 1)]
                z1c = z1p.tile([128, 1024], F32, tag="z1c")
                for j in range(2):
                    nc.tensor.matmul(z1c[:, 512 * j:512 * (j + 1)],
                                     lhsT=w1t[:],
                                     rhs=fq[:, 512 * j:512 * (j + 1)],
                                     start=True, stop=True)
                h1c = h1p.tile([128, 1024], BF16, tag="h1c")
                nc.scalar.activation(out=h1c, in_=z1c[:],
                                     func=mybir.ActivationFunctionType.Relu,
                                     bias=b1t[:, 0:1], scale=1.0)
                z2c = z2p.tile([128, 1024], F32, tag="z2c")
                for j in range(2):
                    nc.tensor.matmul(z2c[:, 512 * j:512 * (j + 1)],
                                     lhsT=w2t[:],
                                     rhs=h1c[:, 512 * j:512 * (j + 1)],
                                     start=True, stop=True)
                if k in copy_chunks:
                    (s, c_lo, c_hi, slot) = chunk_slots[k][0]
                    zc = zcp.tile([128, 1024], F32, tag="zc")
                    nc.scalar.copy(out=zc, in_=z2c[:])
                    w_ = 512
                    while w_ >= 32:
                        nc.gpsimd.tensor_tensor(out=zc[:, 0:w_],
                                                in0=zc[:, 0:w_],
                                                in1=zc[:, w_:2 * w_],
                                                op=mybir.AluOpType.max)
                        w_ //= 2
                    nc.vector.reduce_max(minis[:, slot:slot + 1], zc[:, 0:32],
                                         axis=mybir.AxisListType.X)
                else:
                    for (s, c_lo, c_hi, slot) in chunk_slots[k]:
                        nc.vector.reduce_max(minis[:, slot:slot + 1],
                                             z2c[:, c_lo:c_hi],
                                             axis=mybir.AxisListType.X)
                for s in range(SEGS_PER_CORE):
                    if seg_last_chunk[s] == k:
                        finish_segment_stage1(s)
                    if seg_last_chunk[s] == k - 2:
                        finish_segment_stage2(s)
            for s in range(SEGS_PER_CORE):
                if seg_last_chunk[s] >= NCHUNK - 2:
                    finish_segment_stage2(s)

    nc.compile()
    return nc


_CACHE = {}


def _program(S_cap):
    if S_cap not in _CACHE:
        _CACHE[S_cap] = _build_program(S_cap)
    return _CACHE[S_cap]


def _prep_inputs(pos, reflectance, batch, gumbels, W1, b1, W2, b2, Wg, bg):
    pos = np.asarray(pos, np.float32)
    reflectance = np.asarray(reflectance, np.float32)
    batch = np.asarray(batch, np.int32)
    gumbels = np.asarray(gumbels, np.float32)
    W1, b1 = np.asarray(W1, np.float32), np.asarray(b1, np.float32)
    W2, b2 = np.asarray(W2, np.float32), np.asarray(b2, np.float32)
    Wg, bg = np.asarray(Wg, np.float32), np.asarray(bg, np.float32)

    bounds = np.searchsorted(batch, np.arange(B + 1), side="left")
    seg_len = np.diff(bounds)
    S_cap = int(-(-max(1, seg_len.max()) // 1024) * 1024)
    NCHUNK = 8 * S_cap // CHUNK_PTS
    W = S_cap // 128
    C = 8 * W

    feat = np.concatenate([pos, reflectance[:, None]], axis=1)  # [N,4]

    # constants (shared across cores)
    w1b = np.zeros((32, 128), np.float32)
    w2b = np.zeros((128, 128), np.float32)
    for g in range(8):
        w1b[4 * g:4 * (g + 1), 16 * g:16 * (g + 1)] = W1
        w2b[16 * g:16 * (g + 1), 16 * g:16 * (g + 1)] = W2
    b1r = np.tile(b1, 8)[:, None].astype(np.float32)
    b2r = b2[None, :].astype(np.float32)
    wdr = (Wg[:, 1] - Wg[:, 0])[None, :].astype(np.float32)
    ident = np.eye(128, dtype=np.float32)
    gdel = (bg[1] - bg[0]) + gumbels[:, 1] - gumbels[:, 0]  # [B]

    in_maps = []
    for core in range(NCORES):
        fpad = np.zeros((SEGS_PER_CORE, S_cap, 4), np.float32)
        rpad = np.zeros((SEGS_PER_CORE, S_cap), np.float32)
        for s in range(SEGS_PER_CORE):
            seg = SEGS_PER_CORE * core + s
            lo, hi = bounds[seg], bounds[seg + 1]
            n = hi - lo
            if n > 0:
                fpad[s, :n] = feat[lo:hi]
                fpad[s, n:] = feat[lo]        # replicate first point
                rpad[s, :n] = reflectance[lo:hi]
        # fh[k, 4g+f, c] = feat(core-point 8192k + 8c + g, f)
        fh = (fpad.reshape(8 * S_cap, 4)
                  .reshape(NCHUNK, 1024, 8, 4)   # k, c, g, f
                  .transpose(0, 2, 3, 1)         # k, g, f, c
                  .reshape(NCHUNK * 32, 1024))
        rh = rpad.reshape(C, 128).T               # [128, C], pt = 128c + p
        gdb = gdel[SEGS_PER_CORE * core:SEGS_PER_CORE * (core + 1)][None, :]
        in_maps.append({
            "feat": np.ascontiguousarray(fh.astype(NPBF)),
            "refl": np.ascontiguousarray(rh),
            "gdb": np.ascontiguousarray(gdb.astype(np.float32)),
            "w1b": w1b.astype(NPBF), "w2b": w2b.astype(NPBF),
            "b1r": b1r, "b2r": b2r, "wdr": wdr, "ident": ident,
        })
    return in_maps, bounds, S_cap


_LAST_S_CAP = None


def _run(trace=False, **inputs):
    global _LAST_S_CAP
    in_maps, bounds, S_cap = _prep_inputs(**inputs)
    _LAST_S_CAP = S_cap
    nc = _program(S_cap)
    res = run_bass_kernel_spmd(nc, in_maps, list(range(NCORES)), trace=trace)
    out = np.empty(N, np.float32)
    for core in range(NCORES):
        o = res.results[core]["out"]              # [128, C]
        flat = o.T.reshape(SEGS_PER_CORE, S_cap)  # [s, S_cap]
        for s in range(SEGS_PER_CORE):
            seg = SEGS_PER_CORE * core + s
            lo, hi = bounds[seg], bounds[seg + 1]
            if hi > lo:
                out[lo:hi] = flat[s, :hi - lo]
    return out, res


def kernel(**inputs) -> np.ndarray:
    out, _ = _run(trace=False, **inputs)
    return out


# revision 27
# speedup vs baseline: 1.0478x; 1.0147x over previous
"""Trainium2 Bass kernel for BinaryReflectanceGate (gnn_message_passing).

Math (reference):
    feat = [pos, refl]                    # [N,4]
    h1 = relu(feat @ W1 + b1)             # [N,16]
    h2 = relu(h1 @ W2 + b2)               # [N,16]
    smax = segment_max(h2, batch, B=64)   # [64,16]
    logits = smax @ Wg + bg               # [64,2]
    gate = softmax((logits + gumbels)/tau)[:, 1]
    out = gate[batch] * refl              # [N]

Kernel v2 strategy (8 cores, data parallel over whole samples):
  - batch is sorted; core k owns segments [8k, 8k+8), each padded to a
    uniform S_cap (multiple of 1024; pad points replicate the segment's
    first point so the max is unchanged).
  - column-major group interleave: core-point p -> chunk k = p//8192,
    col c = (p%8192)//8, group g = p%8.  Chunks are [128, 1024] tiles
    (partition = 16g+ch) and segment boundaries fall on column
    boundaries inside a chunk, so boundary chunks split into column
    ranges (no double-cost partition splits).
  - bf16 matmuls (1 cycle/row): L1 via [32,512]-rhs block-diagonal
    weights (8 groups x 4 features), L2 via [128,128] block-diagonal.
  - relu/b1 fused in one ACT activation per chunk, bf16 h1 out.
  - relu/b2 deferred past the segment max (monotone).
  - the per-chunk segment-max runs on DVE (the only engine that can
    column-reduce PSUM).  ACT is cheaper per column than DVE, so a few
    chunks per core take a copy path instead: ACT copies z2 to SBUF,
    Pool (which cannot touch PSUM) max-folds the copy, and a tiny DVE
    reduce finishes the slot.  This shifts work from DVE (the
    bottleneck) onto ACT/Pool headroom.
  - softmax over 2 classes == sigmoid of the logit difference.
  - per-segment finishers: Pool folds minis slots pairwise, PE
    transposes, DVE does the 8-group row max + wd dot, ACT applies the
    sigmoid, Pool broadcasts the gate and scales the reflectance.
"""
import sys
sys.path.insert(0, "/opt/trn_rl_repo")

import numpy as np
import ml_dtypes
import concourse.bass as bass
import concourse.bacc as bacc
from concourse import mybir
from concourse.tile import TileContext
from concourse.bass_utils import run_bass_kernel_spmd

N = 4_194_304
B = 64
H = 16
NCORES = 8
SEGS_PER_CORE = B // NCORES    # 8
CHUNK_PTS = 8192               # points per [128,1024] z chunk
NPBF = ml_dtypes.bfloat16

F32 = mybir.dt.float32
BF16 = mybir.dt.bfloat16

# number of chunks routed ACT-copy + DVE-bf16-fold instead of DVE PSUM reduce
# (measured: ACT copies stall the relu cadence -> keep 0)
N_COPY = 0


def _chunk_regions(S_cap):
    """Per chunk: list of (seg, c_lo, c_hi).  Segment boundaries are at
    core-point s*S_cap; chunk k covers points [8192k, 8192(k+1)) with
    col c = (p - 8192k)//8."""
    nchunk = 8 * S_cap // CHUNK_PTS
    regions = []
    for k in range(nchunk):
        p0, p1 = k * CHUNK_PTS, (k + 1) * CHUNK_PTS
        s0, s1 = p0 // S_cap, (p1 - 1) // S_cap
        if s0 == s1:
            regions.append([(s0, 0, 1024)])
        else:
            m = (s1 * S_cap - p0) // 8
            regions.append([(s0, 0, m), (s1, m, 1024)])
    return regions


def _build_program(S_cap):
    assert S_cap % 1024 == 0
    NCHUNK = 8 * S_cap // CHUNK_PTS
    W = S_cap // 128            # refl/out columns per segment
    C = 8 * W                   # [128, C] refl/out

    regions = _chunk_regions(S_cap)
    # choose copy chunks: single-segment chunks away from segment ends
    seg_last_chunk = [0] * SEGS_PER_CORE
    for k, regs in enumerate(regions):
        for (s, _, _) in regs:
            seg_last_chunk[s] = k
    copy_ok = [k for k, regs in enumerate(regions)
               if len(regs) == 1 and k + 3 < seg_last_chunk[regs[0][0]] and k > 1]
    stride = max(1, len(copy_ok) // max(1, N_COPY))
    copy_chunks = set(copy_ok[stride // 2::stride][:N_COPY])

    # slot assignment (per region, sequential -> per-seg slots contiguous)
    chunk_slots = []            # per chunk: list of (seg, c_lo, c_hi, slot)
    seg_slots = [[] for _ in range(SEGS_PER_CORE)]
    nslot = 0
    for k, regs in enumerate(regions):
        out = []
        for (s, c_lo, c_hi) in regs:
            out.append((s, c_lo, c_hi, nslot))
            seg_slots[s].append(nslot)
            nslot += 1
        chunk_slots.append(out)
    for s in range(SEGS_PER_CORE):
        sl = seg_slots[s]
        assert sl == list(range(sl[0], sl[-1] + 1))

    nc = bacc.Bacc()

    feat_d = nc.declare_dram_parameter("feat", [NCHUNK * 32, 1024], BF16,
                                       isOutput=False)
    refl_d = nc.declare_dram_parameter("refl", [128, C], F32, isOutput=False)
    gdb_d = nc.declare_dram_parameter("gdb", [1, 8], F32, isOutput=False)
    # wblob: cols 0-127 = w2 block-diag, 128-255 = w1 block-diag (rows 0-31)
    wblob_d = nc.declare_dram_parameter("wblob", [128, 256], BF16,
                                        isOutput=False)
    b1_d = nc.declare_dram_parameter("b1r", [128, 1], F32, isOutput=False)
    b2_d = nc.declare_dram_parameter("b2r", [1, 16], F32, isOutput=False)
    wd_d = nc.declare_dram_parameter("wdr", [1, 16], F32, isOutput=False)
    id_d = nc.declare_dram_parameter("ident", [128, 128], F32, isOutput=False)
    out_d = nc.declare_dram_parameter("out", [128, C], F32, isOutput=True)

    with TileContext(nc) as tc:
        with tc.tile_pool(name="consts", bufs=1) as consts, \
             tc.tile_pool(name="big", bufs=1) as big, \
             tc.tile_pool(name="feat", bufs=3) as featp, \
             tc.tile_pool(name="h1", bufs=5) as h1p, \
             tc.tile_pool(name="zc", bufs=2) as zcp, \
             tc.tile_pool(name="fin", bufs=1) as fin, \
             tc.tile_pool(name="z1", bufs=2, space="PSUM") as z1p, \
             tc.tile_pool(name="z2", bufs=2, space="PSUM") as z2p:

            wblob = consts.tile([128, 256], BF16)
            w2t = wblob[:, 0:128]
            w1t = wblob[0:32, 128:256]
            b1t = consts.tile([128, 1], F32)
            b2r = consts.tile([1, 16], F32)
            wdr = consts.tile([1, 16], F32)
            gdbt = consts.tile([1, 8], F32)
            ident = consts.tile([128, 128], F32)

            reflt = big.tile([128, C], F32)
            outt = big.tile([128, C], F32)

            minis = fin.tile([128, nslot], F32)
            nc.gpsimd.memset(minis, -1e30)
            # preload the ACT table set (sigmoid set also contains relu
            # and copy) before the first real activation
            preact = fin.tile([1, 1], F32)
            nc.vector.memset(preact, 0.0)
            nc.scalar.activation(out=preact, in_=preact[:],
                                 func=mybir.ActivationFunctionType.Sigmoid,
                                 bias=0.0, scale=1.0)
            nc.scalar.activation(out=preact, in_=preact[:],
                                 func=mybir.ActivationFunctionType.Relu,
                                 bias=0.0, scale=1.0)

            seg_red = {}

            def finish_segment_stage1(s):
                # fold this segment's minis slots into one column (DVE),
                # transpose to a row (PE), stage the row in SBUF (ACT)
                lo, hi = seg_slots[s][0], seg_slots[s][-1] + 1
                red = fin.tile([128, 1], F32, tag=f"red{s}")
                nc.vector.reduce_max(red, minis[:, lo:hi],
                                     axis=mybir.AxisListType.X)
                tp = z2p.tile([1, 128], F32, tag="z2c")
                nc.tensor.transpose(tp, red[:], ident[:])
                seg_red[s] = tp

            def finish_segment_stage2(s):
                # 8-group row max + gate + reflectance scaling
                tp = seg_red[s]
                tail = s == SEGS_PER_CORE - 1
                teng = nc.vector if tail else nc.gpsimd
                row16 = fin.tile([1, 16], F32, tag=f"row{s}")
                nc.vector.reduce_max(
                    row16, tp.rearrange("one (g ch) -> one ch g", g=8),
                    axis=mybir.AxisListType.X)
                srel = fin.tile([1, 16], F32, tag=f"srel{s}")
                teng.tensor_tensor(srel, row16, b2r[:],
                                   op=mybir.AluOpType.add)
                teng.tensor_scalar_max(srel, srel, 0.0)
                teng.tensor_mul(srel, srel, wdr[:])
                logit = fin.tile([1, 1], F32, tag=f"lg{s}")
                nc.vector.reduce_sum(logit, srel, axis=mybir.AxisListType.X)
                gate1 = fin.tile([1, 1], F32, tag=f"g{s}")
                nc.scalar.activation(out=gate1, in_=logit[:],
                                     func=mybir.ActivationFunctionType.Sigmoid,
                                     bias=gdbt[0:1, s:s + 1], scale=1.0)
                gbc = fin.tile([128, 1], F32, tag=f"gb{s}")
                nc.gpsimd.partition_broadcast(gbc, gate1[:])
                if s == SEGS_PER_CORE - 1:
                    # tail-exposed: split so the first out DMA overlaps
                    # the second half's multiply
                    h = W // 2
                    for lo, hi in ((W * s, W * s + h), (W * s + h, W * (s + 1))):
                        nc.vector.tensor_scalar_mul(outt[:, lo:hi],
                                                    reflt[:, lo:hi],
                                                    gbc[:, 0:1])
                        nc.sync.dma_start(out=out_d[:, lo:hi],
                                          in_=outt[:, lo:hi])
                else:
                    nc.gpsimd.tensor_scalar_mul(outt[:, W * s:W * (s + 1)],
                                                reflt[:, W * s:W * (s + 1)],
                                                gbc[:, 0:1])
                    nc.sync.dma_start(out=out_d[:, W * s:W * (s + 1)],
                                      in_=outt[:, W * s:W * (s + 1)])

            # critical consts first: one blob DMA gates both matmul layers
            nc.sync.dma_start(out=wblob, in_=wblob_d[:])

            refl_dma_done = set()
            ft = None
            h1_tiles = {}
            z2_of = {}

            def emit_front(k):
                """DMA + L1 + relu for chunk k."""
                nonlocal ft
                if k == 0:
                    # chunk 0 alone so the first matmul starts early
                    ft = featp.tile([32, 4096], BF16, tag="ft")
                    nc.sync.dma_start(
                        out=ft[:, 0:1024],
                        in_=bass.AP(tensor=feat_d.ap().tensor, offset=0,
                                    ap=[[1024, 32], [1, 1024]]))
                    nc.sync.dma_start(out=b1t, in_=b1_d[:])
                    nc.sync.dma_start(
                        out=ft[:, 1024:4096],
                        in_=bass.AP(tensor=feat_d.ap().tensor, offset=32768,
                                    ap=[[1024, 32], [32768, 3], [1, 1024]]))
                elif k % 4 == 0:
                    nq = min(4, NCHUNK - k)
                    ft = featp.tile([32, 4096], BF16, tag="ft")
                    nc.sync.dma_start(
                        out=ft[:, 0:1024 * nq],
                        in_=bass.AP(tensor=feat_d.ap().tensor,
                                    offset=k * 32768,
                                    ap=[[1024, 32], [32768, nq], [1, 1024]]))
                if k == 1:
                    nc.sync.dma_start(out=ident, in_=id_d[:])
                    nc.sync.dma_start(out=b2r, in_=b2_d[:])
                    nc.sync.dma_start(out=wdr, in_=wd_d[:])
                    nc.sync.dma_start(out=gdbt, in_=gdb_d[:])
                # spread the reflectance loads across the loop (2 segs per DMA)
                s_here = (k * CHUNK_PTS) // (2 * S_cap)
                if s_here not in refl_dma_done:
                    refl_dma_done.add(s_here)
                    nc.sync.dma_start(
                        out=reflt[:, 2 * W * s_here:2 * W * (s_here + 1)],
                        in_=refl_d[:, 2 * W * s_here:2 * W * (s_here + 1)])

                fq = ft[:, 1024 * (k % 4):1024 * (k % 4 + 1)]
                z1c = z1p.tile([128, 1024], F32, tag="z1c")
                for j in range(2):
                    nc.tensor.matmul(z1c[:, 512 * j:512 * (j + 1)],
                                     lhsT=w1t[:],
                                     rhs=fq[:, 512 * j:512 * (j + 1)],
                                     start=True, stop=True)
                h1c = h1p.tile([128, 1024], BF16, tag="h1c")
                nc.scalar.activation(out=h1c, in_=z1c[:],
                                     func=mybir.ActivationFunctionType.Relu,
                                     bias=b1t[:, 0:1], scale=1.0)
                h1_tiles[k] = h1c

            pending_folds = []

            def emit_pending_folds():
                while pending_folds:
                    zcb, slot = pending_folds.pop(0)
                    for w_ in (512, 256, 128):
                        nc.vector.tensor_tensor(out=zcb[:, 0:w_],
                                                in0=zcb[:, 0:w_],
                                                in1=zcb[:, w_:2 * w_],
                                                op=mybir.AluOpType.max)
                    nc.vector.reduce_max(minis[:, slot:slot + 1],
                                         zcb[:, 0:128],
                                         axis=mybir.AxisListType.X)

            def emit_back(k):
                """L2 + reduce + finishers for chunk k."""
                h1c = h1_tiles.pop(k)
                z2c = z2p.tile([128, 1024], F32, tag="z2c")
                for j in range(2):
                    nc.tensor.matmul(z2c[:, 512 * j:512 * (j + 1)],
                                     lhsT=w2t[:],
                                     rhs=h1c[:, 512 * j:512 * (j + 1)],
                                     start=True, stop=True)
                if k in copy_chunks:
                    # ACT egress to SBUF bf16; DVE folds (2x bf16) deferred
                    # one chunk so the in-order DVE queue never waits on ACT
                    (s, c_lo, c_hi, slot) = chunk_slots[k][0]
                    zcb = zcp.tile([128, 1024], BF16, tag="zc")
                    nc.scalar.copy(out=zcb, in_=z2c[:])
                    pending_folds.append((zcb, slot))
                else:
                    for (s, c_lo, c_hi, slot) in chunk_slots[k]:
                        nc.vector.reduce_max(minis[:, slot:slot + 1],
                                             z2c[:, c_lo:c_hi],
                                             axis=mybir.AxisListType.X)
                    emit_pending_folds()
                for s in range(SEGS_PER_CORE):
                    if seg_last_chunk[s] == k:
                        finish_segment_stage1(s)
                    if seg_last_chunk[s] == k - 2:
                        finish_segment_stage2(s)

            LEAD = 1   # L1 runs this many chunks ahead of L2
            for k in range(NCHUNK + LEAD):
                if k < NCHUNK:
                    emit_front(k)
                if k >= LEAD:
                    emit_back(k - LEAD)
            for s in range(SEGS_PER_CORE):
                if seg_last_chunk[s] >= NCHUNK - 2:
                    finish_segment_stage2(s)

    nc.compile()
    return nc


_CACHE = {}


def _program(S_cap):
    if S_cap not in _CACHE:
        _CACHE[S_cap] = _build_program(S_cap)
    return _CACHE[S_cap]


def _prep_inputs(pos, reflectance, batch, gumbels, W1, b1, W2, b2, Wg, bg):
    pos = np.asarray(pos, np.float32)
    reflectance = np.asarray(reflectance, np.float32)
    batch = np.asarray(batch, np.int32)
    gumbels = np.asarray(gumbels, np.float32)
    W1, b1 = np.asarray(W1, np.float32), np.asarray(b1, np.float32)
    W2, b2 = np.asarray(W2, np.float32), np.asarray(b2, np.float32)
    Wg, bg = np.asarray(Wg, np.float32), np.asarray(bg, np.float32)

    bounds = np.searchsorted(batch, np.arange(B + 1), side="left")
    seg_len = np.diff(bounds)
    S_cap = int(-(-max(1, seg_len.max()) // 1024) * 1024)
    NCHUNK = 8 * S_cap // CHUNK_PTS
    W = S_cap // 128
    C = 8 * W

    feat = np.concatenate([pos, reflectance[:, None]], axis=1)  # [N,4]

    # constants (shared across cores)
    w1b = np.zeros((32, 128), np.float32)
    w2b = np.zeros((128, 128), np.float32)
    for g in range(8):
        w1b[4 * g:4 * (g + 1), 16 * g:16 * (g + 1)] = W1
        w2b[16 * g:16 * (g + 1), 16 * g:16 * (g + 1)] = W2
    wblob = np.zeros((128, 256), np.float32)
    wblob[:, 0:128] = w2b
    wblob[0:32, 128:256] = w1b
    b1r = np.tile(b1, 8)[:, None].astype(np.float32)
    b2r = b2[None, :].astype(np.float32)
    wdr = (Wg[:, 1] - Wg[:, 0])[None, :].astype(np.float32)
    ident = np.eye(128, dtype=np.float32)
    gdel = (bg[1] - bg[0]) + gumbels[:, 1] - gumbels[:, 0]  # [B]

    in_maps = []
    for core in range(NCORES):
        fpad = np.zeros((SEGS_PER_CORE, S_cap, 4), np.float32)
        rpad = np.zeros((SEGS_PER_CORE, S_cap), np.float32)
        for s in range(SEGS_PER_CORE):
            seg = SEGS_PER_CORE * core + s
            lo, hi = bounds[seg], bounds[seg + 1]
            n = hi - lo
            if n > 0:
                fpad[s, :n] = feat[lo:hi]
                fpad[s, n:] = feat[lo]        # replicate first point
                rpad[s, :n] = reflectance[lo:hi]
        # fh[k, 4g+f, c] = feat(core-point 8192k + 8c + g, f)
        fh = (fpad.reshape(8 * S_cap, 4)
                  .reshape(NCHUNK, 1024, 8, 4)   # k, c, g, f
                  .transpose(0, 2, 3, 1)         # k, g, f, c
                  .reshape(NCHUNK * 32, 1024))
        rh = rpad.reshape(C, 128).T               # [128, C], pt = 128c + p
        gdb = gdel[SEGS_PER_CORE * core:SEGS_PER_CORE * (core + 1)][None, :]
        in_maps.append({
            "feat": np.ascontiguousarray(fh.astype(NPBF)),
            "refl": np.ascontiguousarray(rh),
            "gdb": np.ascontiguousarray(gdb.astype(np.float32)),
            "wblob": wblob.astype(NPBF),
            "b1r": b1r, "b2r": b2r, "wdr": wdr, "ident": ident,
        })
    return in_maps, bounds, S_cap


_LAST_S_CAP = None


def _run(trace=False, **inputs):
    global _LAST_S_CAP
    in_maps, bounds, S_cap = _prep_inputs(**inputs)
    _LAST_S_CAP = S_cap
    nc = _program(S_cap)
    res = run_bass_kernel_spmd(nc, in_maps, list(range(NCORES)), trace=trace)
    out = np.empty(N, np.float32)
    for core in range(NCORES):
        o = res.results[core]["out"]              # [128, C]
        flat = o.T.reshape(SEGS_PER_CORE, S_cap)  # [s, S_cap]
        for s in range(SEGS_PER_CORE):
            seg = SEGS_PER_CORE * core + s
            lo, hi = bounds[seg], bounds[seg + 1]
            if hi > lo:
                out[lo:hi] = flat[s, :hi - lo]
    return out, res


def kernel(**inputs) -> np.ndarray:
    out, _ = _run(trace=False, **inputs)
    return out
